# revision 5
# baseline (speedup 1.0000x reference)
"""Trainium2 Bass kernel for nn_LowRankRedistributor (8-core SPMD).

Math: with s = ||logits||^2, h = logits @ W, d = ||h||^2, m = h @ W.T,
t = ||m||^2, q = 0.58 + 0.42*d/sqrt(t*s):
    out = alpha*m + beta*logits,  alpha = 0.7*sqrt(s/(t*q)),  beta = 0.3/sqrt(q)
(all the reference's normalizations fold into the two scalars).

Sharding: vocab (V=128000) split over 8 cores, 16000 rows of W each.
Core c computes h_c = logits_c @ W_c (AllReduce h + s), then its vocab
shard m_c = h @ W_c.T (AllReduce t), then the lerp, all on-chip.
W is fed as fp16 (halves DMA; 11-bit mantissa keeps rel err ~3e-4);
accumulation is fp32 in PSUM. Host pre-swizzles inputs so every DMA is
contiguous >=512B per partition; output leaves v-major [128, 1000] per
core and is un-swizzled on the host.
"""
import sys, os, types

sys.path.insert(0, "/opt/trn_rl_repo")

import numpy as np

# --- NTFF profiling hook (the agent image's antenv lacks axon_hooks) ---
import antenv  # noqa: E402

if "antenv.axon_hooks" not in sys.modules:
    _hooks = types.ModuleType("antenv.axon_hooks")
    _h = [None]
    _hooks.set_axon_ntff_profile_hook = lambda h: _h.__setitem__(0, h)
    _hooks.get_axon_ntff_profile_hook = lambda: _h[0]
    sys.modules["antenv.axon_hooks"] = _hooks
    antenv.axon_hooks = _hooks
    try:
        from trn_agent_boot.trn_boot import _ntff_profile_via_ctypes
        _hooks.set_axon_ntff_profile_hook(
            _ntff_profile_via_ctypes("/opt/axon/libaxon_pjrt.so"))
    except Exception:
        pass

from concourse import bacc, tile, mybir, bass_utils  # noqa: E402
from concourse import bass_isa  # noqa: E402
from concourse.bass_interp import get_hw_module  # noqa: E402

N_CORES = 8
B, V, R = 8, 128000, 512
VC = V // N_CORES          # 16000 vocab rows per core
NT = VC // 128             # 125 v-tiles of 128
NJ = R // 128              # 4 r-chunks of 128
F16 = mybir.dt.float16
F32 = mybir.dt.float32

_CACHED_NC = None


def _strip_pseudo(nc):
    import copy as _copy
    new_module = _copy.replace(nc.m, functions=[])
    for function in nc.m.functions:
        new_function = _copy.replace(function, blocks=[])
        new_function.set_allocations_from_list(function.allocations)
        for block in function.blocks:
            new_function.blocks.append(_copy.replace(
                block,
                instructions=[
                    i for i in block.instructions
                    if not isinstance(i, (mybir.BassTilePoolBoundary,
                                          mybir.BassTileRelease))],
            ))
        new_module.functions.append(new_function)
    nc.m = new_module


def build():
    nc = bacc.Bacc("TRN2", target_bir_lowering=False, debug=False,
                   num_devices=N_CORES)
    Add = mybir.AluOpType
    Act = mybir.ActivationFunctionType

    w16 = nc.dram_tensor("w16", [128, NT, 512], F16, kind="ExternalInput").ap()
    wt16 = nc.dram_tensor("wt16", [128, NJ, VC], F16, kind="ExternalInput").ap()
    lt16 = nc.dram_tensor("lt16", [128, NT, B], F16, kind="ExternalInput").ap()
    lgvm = nc.dram_tensor("lgvm", [128, NT * B], F32, kind="ExternalInput").ap()
    id8_d = nc.inline_tensor(np.eye(8, dtype=np.float32), name="id8").ap()
    ones_c_d = nc.inline_tensor(np.ones((128, 1), np.float32), name="ones_c").ap()
    ones_r_d = nc.inline_tensor(np.ones((1, 128), np.float32), name="ones_r").ap()
    out_d = nc.dram_tensor("out", [128, NT * B], F32, kind="ExternalOutput").ap()

    with tile.TileContext(nc) as tc, \
         tc.tile_pool(name="persist", bufs=1) as pp, \
         tc.tile_pool(name="persist_ps", bufs=1, space="PSUM") as ps, \
         tc.tile_pool(name="persist_dr", bufs=1, space="DRAM") as dr:
        # ---------------- persistent SBUF tensors ----------------
        lt_sb = pp.tile([128, NT, B], F16, name="lt_sb")
        lg_sb = pp.tile([128, NT * B], F32, name="lg_sb")
        sqscr = pp.tile([128, NT * B], F32, name="sqscr")
        s_vp = pp.tile([128, 1], F32, name="s_vp")
        id8 = pp.tile([8, 8], F32, name="id8sb")
        hs_in = pp.tile([8, 513], F32, name="hs_in")
        hg = pp.tile([8, 513], F32, name="hg")
        hT16 = pp.tile([128, NJ, B], F16, name="hT16")
        dscr = pp.tile([8, 512], F32, name="dscr")
        d_row = pp.tile([8, 1], F32, name="d_row")
        m_vm = pp.tile([128, NT * B], F32, name="m_vm")
        t_cols = pp.tile([8, 32], F32, name="t_cols")
        t_row = pp.tile([8, 1], F32, name="t_row")
        tg = pp.tile([8, 1], F32, name="tg")
        sc = pp.tile([1, 16], F32, name="sc")
        ones_c = pp.tile([128, 1], F32, name="ones_c_sb")
        ones_r = pp.tile([1, 128], F32, name="ones_r_sb")
        ab = pp.tile([1, 2], F32, name="ab")
        ab_bc = pp.tile([128, 2], F32, name="ab_bc")
        lgs = pp.tile([128, NT * B], F32, name="lgs")

        # collective bounce buffers (DRAM)
        ar1_in = dr.tile([8, 513], F32, name="ar1_in")
        ar1_out = dr.tile([8, 513], F32, addr_space="Shared", name="ar1_out")
        ar2_in = dr.tile([8, 1], F32, name="ar2_in")
        ar2_out = dr.tile([8, 1], F32, addr_space="Shared", name="ar2_out")

        # PSUM
        psum_h = ps.tile([8, 512], F32, name="psum_h")
        ps_sc = ps.tile([1, 4], F32, name="ps_sc")
        ps_bc = ps.tile([128, 2], F32, name="ps_bc")

        # small loads
        nc.sync.dma_start(out=lt_sb[:, :, :], in_=lt16[:, :, :])
        nc.sync.dma_start(out=lg_sb[:, :], in_=lgvm[:, :])
        nc.sync.dma_start(out=id8[:, :], in_=id8_d[:, :])
        nc.sync.dma_start(out=ones_c[:, :], in_=ones_c_d[:, :])
        nc.sync.dma_start(out=ones_r[:, :], in_=ones_r_d[:, :])

        # ---------------- pass A: h_c = logits_c @ W_c ----------------
        with tc.tile_pool(name="wpool", bufs=4) as wpool:
            done = 0
            si = 0
            while done < NT:
                n = min(8, NT - done)
                wtile = wpool.tile([128, n, 512], F16, tag="w")
                nc.sync.dma_start(out=wtile[:, :, :],
                                  in_=w16[:, done:done + n, :])
                for t in range(n):
                    i = done + t
                    nc.tensor.matmul(psum_h[:, :], lt_sb[:, i, :],
                                     wtile[:, t, :],
                                     start=(i == 0), stop=(i == NT - 1))
                done += n
                si += 1

        # s_c = sum(logits_c^2) via ACT square with accumulate
        nc.scalar.activation(sqscr[:, :], lg_sb[:, :], Act.Square,
                             accum_out=s_vp[:, :])
        nc.tensor.matmul(ps_sc[:, 0:1], ones_c[:, :], s_vp[:, :])

        # assemble AR1 payload [8, 513] = [h_c | s_c(row0)]
        nc.scalar.copy(hs_in[:, 0:512], psum_h[:, :])
        nc.vector.memset(hs_in[:, 512:513], 0.0)
        nc.vector.tensor_copy(hs_in[0:1, 512:513], ps_sc[0:1, 0:1])
        nc.sync.dma_start(out=ar1_in[:, :], in_=hs_in[:, :])
        nc.gpsimd.collective_compute(
            "AllReduce", Add.add, replica_groups=[list(range(N_CORES))],
            ins=[ar1_in.opt()], outs=[ar1_out.opt()])
        nc.sync.dma_start(out=hg[:, :], in_=ar1_out[:, :])

        # d = ||h||^2 (global h), and hT16 = h^T cast to fp16
        nc.scalar.activation(dscr[:, :], hg[:, 0:512], Act.Square,
                             accum_out=d_row[:, :])
        nc.tensor.matmul(ps_sc[:, 1:2], ones_c[0:8, :], d_row[:, :])
        nc.vector.tensor_copy(sc[:, 10:11], ps_sc[:, 1:2])

        psum_ht = ps.tile([128, NJ * B], F32, name="psum_ht")
        for j in range(NJ):
            nc.tensor.transpose(psum_ht[:, j * B:(j + 1) * B],
                                hg[:, 128 * j:128 * (j + 1)], id8[:, :])
        for j in range(NJ):
            nc.vector.tensor_copy(hT16[:, j, :], psum_ht[:, j * B:(j + 1) * B])

        # ---------------- pass B: m_c = h @ W_c.T ----------------
        chunks = []
        v0 = 0
        while v0 < VC:
            chunks.append((v0, min(512, VC - v0)))
            v0 += 512

        with tc.tile_pool(name="wtpool", bufs=8) as wtpool, \
             tc.tile_pool(name="pmpool", bufs=2, space="PSUM") as pmpool, \
             tc.tile_pool(name="ptpool", bufs=2, space="PSUM") as ptpool, \
             tc.tile_pool(name="mbpool", bufs=3) as mbpool:
            for ci, (v0, n) in enumerate(chunks):
                wtt = wtpool.tile([128, NJ, n], F16, tag="wt")
                nc.sync.dma_start(out=wtt[:, :, :], in_=wt16[:, :, v0:v0 + n])
                pm = pmpool.tile([8, n], F32, tag="pm")
                for j in range(NJ):
                    nc.tensor.matmul(pm[:, :], hT16[:, j, :], wtt[:, j, :],
                                     start=(j == 0), stop=(j == NJ - 1))
                mb = mbpool.tile([8, n], F32, tag="mb")
                nc.scalar.copy(mb[:, :], pm[:, :])
                # t partial for this chunk
                sq = mbpool.tile([8, n], F32, tag="sq")
                nc.scalar.activation(sq[:, :], pm[:, :], Act.Square,
                                     accum_out=t_cols[:, ci:ci + 1])
                # transpose to v-major [128, w, b]
                nt = n // 128
                pt = ptpool.tile([128, nt * B], F32, tag="pt")
                for i in range(nt):
                    nc.tensor.transpose(pt[:, i * B:(i + 1) * B],
                                        mb[:, 128 * i:128 * (i + 1)],
                                        id8[:, :])
                col = (v0 // 128) * B
                nc.vector.tensor_copy(m_vm[:, col:col + nt * B], pt[:, :])

        nc.vector.tensor_reduce(t_row[:, :], t_cols[:, :],
                                axis=mybir.AxisListType.X, op=Add.add)
        nc.sync.dma_start(out=ar2_in[:, :], in_=t_row[:, :])
        nc.gpsimd.collective_compute(
            "AllReduce", Add.add, replica_groups=[list(range(N_CORES))],
            ins=[ar2_in.opt()], outs=[ar2_out.opt()])
        nc.sync.dma_start(out=tg[:, :], in_=ar2_out[:, :])
        nc.tensor.matmul(ps_sc[:, 2:3], ones_c[0:8, :], tg[:, :])
        nc.vector.tensor_copy(sc[:, 11:12], ps_sc[:, 2:3])

        # ---------------- scalars ----------------
        nc.vector.tensor_copy(sc[:, 9:10], hg[0:1, 512:513])
        s_ = sc[:, 9:10]
        d_ = sc[:, 10:11]
        t_ = sc[:, 11:12]
        nc.vector.tensor_tensor(sc[:, 0:1], s_, t_, Add.mult)          # u = s*t
        nc.scalar.sqrt(sc[:, 1:2], sc[:, 0:1])                          # su
        nc.vector.reciprocal(sc[:, 2:3], sc[:, 1:2])                    # ru
        nc.vector.tensor_tensor(sc[:, 3:4], d_, sc[:, 2:3], Add.mult)  # w
        nc.scalar.activation(sc[:, 4:5], sc[:, 3:4], Act.Copy,
                             bias=0.58, scale=0.42)                     # q
        nc.vector.reciprocal(sc[:, 5:6], sc[:, 4:5])                    # rq
        nc.scalar.activation(ab[:, 1:2], sc[:, 5:6], Act.Sqrt,
                             scale=0.09)                                # beta
        nc.vector.reciprocal(sc[:, 6:7], t_)                            # rt
        nc.vector.tensor_tensor(sc[:, 7:8], s_, sc[:, 5:6], Add.mult)
        nc.vector.tensor_tensor(sc[:, 8:9], sc[:, 7:8], sc[:, 6:7], Add.mult)
        nc.scalar.activation(ab[:, 0:1], sc[:, 8:9], Act.Sqrt,
                             scale=0.49)                                # alpha
        nc.tensor.matmul(ps_bc[:, :], ones_r[:, :], ab[:, :])
        nc.vector.tensor_copy(ab_bc[:, :], ps_bc[:, :])

        # ---------------- final lerp, v-major ----------------
        nc.vector.tensor_scalar_mul(m_vm[:, :], m_vm[:, :], ab_bc[:, 0:1])
        nc.scalar.activation(lgs[:, :], lg_sb[:, :], Act.Copy,
                             scale=ab_bc[:, 1:2])
        nc.vector.tensor_tensor(m_vm[:, :], m_vm[:, :], lgs[:, :], Add.add)
        nc.sync.dma_start(out=out_d[:, :], in_=m_vm[:, :])

    nc.compile()
    _strip_pseudo(nc)
    nc.m = get_hw_module(nc.m)
    return nc


def _get_nc():
    global _CACHED_NC
    if _CACHED_NC is None:
        _CACHED_NC = build()
    return _CACHED_NC


def _make_in_maps(logits, W):
    logits = np.asarray(logits, np.float32)
    W = np.asarray(W, np.float32)
    in_maps = []
    for c in range(N_CORES):
        Wc = W[VC * c:VC * (c + 1)]
        W16 = Wc.astype(np.float16)
        Lc = logits[:, VC * c:VC * (c + 1)]
        LcT = np.ascontiguousarray(Lc.T)
        in_maps.append({
            "w16": np.ascontiguousarray(
                W16.reshape(NT, 128, 512).transpose(1, 0, 2)),
            "wt16": np.ascontiguousarray(
                W16.T.reshape(NJ, 128, VC).transpose(1, 0, 2)),
            "lt16": np.ascontiguousarray(
                LcT.astype(np.float16).reshape(NT, 128, B).transpose(1, 0, 2)),
            "lgvm": np.ascontiguousarray(
                LcT.reshape(NT, 128, B).transpose(1, 0, 2)).reshape(128, NT * B),
        })
    return in_maps


def _unswizzle(results):
    out = np.empty((B, V), np.float32)
    for c in range(N_CORES):
        arr = results[c]["out"].reshape(128, NT, B)
        out[:, VC * c:VC * (c + 1)] = arr.transpose(2, 1, 0).reshape(B, VC)
    return out


def run(logits, W, trace=False):
    nc = _get_nc()
    in_maps = _make_in_maps(logits, W)
    res = bass_utils.run_bass_kernel_spmd(
        nc, in_maps, core_ids=list(range(N_CORES)), trace=trace)
    return _unswizzle(res.results), res


def kernel(logits, W):
    out, _ = run(logits, W, trace=False)
    return out
